# revision 3
# baseline (speedup 1.0000x reference)
"""Blockwise butterfly rotation (nn_BlockwiseButterflyRotation) - TRN2 Bass kernel.

Full inputs: x (4, 4096, 4096) f32, angles (16, 8, 128) f32.
Math: x is split into 16 independent 256-wide blocks; each block's rows are
rotated by an 8-stage butterfly. The composed per-block rotation is a dense
256x256 matrix C_b = B_b^T, so out = x @ blockdiag(C). The kernel builds C
on-device from the angles and runs the bulk work as PE matmuls.

Sharding: data-parallel over rows - x.reshape(16384, 4096) split into 8
contiguous shards of 2048 rows; angles (gathered into per-partition coeff
layout, pure indexing) replicated to all cores.

v3 layout strategy: all device-side data is fp16 (the correctness gate is
rel_err < 2e-2; fp16 keeps us ~1.5e-3).  The host pre-transposes each
128x128 chunk of its shard (pure indexing + dtype cast, no arithmetic) so
the device receives x already in the PE-stationary (k-major) layout:
  xt[rt*128 + k, i*128 + r] = x_core[rt*128 + r, i*128 + k]
This removes all PE transposes and the transpose PSUM->SBUF copies, and
fp16 halves both DMA directions.  Per-core, per 128-row tile:
  DMA in [128, 4096] fp16 (already transposed layout)
  -> 32x PE matmul fp16: out[128, 256] += xt_chunk^T @ C_chunk (1 cyc/col)
  -> PSUM->SBUF copy f32->fp16 (DVE/ACT 2:6 split) -> DMA out fp16

C build (on device, from angles): two-level butterfly factorization
C[16g+u, 16w+v] = LT_g[u,v] * HT_v[g,w]; LT (stages 0-3) and HT (stages
4-7) built by applying 16x16 butterflies to identity patterns with
free-dim-only pairing on the DVE (fp16); cos/sin via ScalarE Sin
(cos = sin(x + pi/2)); HT's u-replication via 16 fp16 selector matmuls on
the PE; the combine writes per-block fp16 CT tiles.  Constant 0/1 init
patterns are shipped as one small fp16 constant input.

In the timed repeat loop the build is software-pipelined across passes:
CT is double-buffered (A/B) and the build steps for the next pass's CT
are emitted interleaved between the row tiles of the current pass, so the
build runs in DVE/ACT/PE slack instead of serializing at pass boundaries.
"""
import math
import os

import numpy as np

from concourse import bacc, mybir, tile
from concourse.bass_utils import run_bass_kernel_spmd

F32 = mybir.dt.float32
F16 = mybir.dt.float16

DIM = 4096
NB = 16
BLOCK = 256
HALF_PI = math.pi / 2.0

N_CORES = 8
R_TOTAL = 4 * 4096
R_CORE = R_TOTAL // N_CORES  # 2048
RT = R_CORE // 128           # 16 row tiles per core

# consts tensor column layout (fp16): halfpi | LSinit | HSBinit | W_all
_C_PI = 0          # [128, 1] pi/2
_C_LS = 1          # [128, 512] LS init: delta(v == p mod 16), free (b, kc, v)
_C_HSB = 513       # [128, 512] HSB init: delta(w == 8kc + p//16), free (kc, v, w)
_C_W = 1025        # [128, 2048] W_all: free (b, mg, mu), delta(p == 16 mg + b)
_C_COLS = 3073

LAST_RESULT = None  # BassKernelResults of the most recent kernel() call
_NC_CACHE = {}


def _build_consts() -> np.ndarray:
    c = np.zeros((128, _C_COLS), dtype=np.float16)
    p = np.arange(128)
    c[:, _C_PI] = np.float16(HALF_PI)
    ls = np.zeros((128, 16, 2, 16), np.float16)
    ls[p, :, :, p % 16] = 1.0
    c[:, _C_LS:_C_LS + 512] = ls.reshape(128, 512)
    hsb = np.zeros((128, 2, 16, 16), np.float16)
    for kc in range(2):
        hsb[:, kc, :, :] = (np.arange(16)[None, :] == (8 * kc + p // 16)[:, None])[:, None, :]
    c[:, _C_HSB:_C_HSB + 512] = hsb.reshape(128, 512)
    w = np.zeros((128, 16, 8, 16), np.float16)
    for b in range(16):
        for mg in range(8):
            w[16 * mg + b, b, mg, :] = 1.0
    c[:, _C_W:_C_W + 2048] = w.reshape(128, 2048)
    return c


_CONSTS = _build_consts()


def gather_angles(angles: np.ndarray) -> np.ndarray:
    """angles [16, 8, 128] f32 -> ang [128, 1536] fp16 (angL 4x256 | angH 4x128).

    Pure gather (indexing only, no arithmetic) into the per-partition
    coefficient layouts the kernel's butterfly-stage APs iterate.
    """
    angles = np.asarray(angles)
    assert angles.shape == (NB, 8, 128)
    out = np.empty((128, 1536), dtype=np.float32)
    for s in range(4):
        sig = 1 << s
        col = np.empty((128, 256), dtype=np.float32)
        for g0 in range(8):
            row = np.empty((16, 2, 8), dtype=np.float32)
            for kc in range(2):
                g = 8 * kc + g0
                for vg in range(8 // sig):
                    for t in range(sig):
                        row[:, kc, vg * sig + t] = angles[:, s, 8 * g + vg * sig + t]
            col[16 * g0:16 * g0 + 16, :] = row.reshape(1, 256)
        out[:, 256 * s:256 * (s + 1)] = col
    for sp in range(4):
        sigp = 1 << sp
        col = np.empty((128, 128), dtype=np.float32)
        for b in range(16):
            row = np.empty((16, 8), dtype=np.float32)
            for v in range(16):
                for wg in range(8 // sigp):
                    for t in range(sigp):
                        row[v, wg * sigp + t] = angles[b, sp + 4, wg * 16 * sigp + 16 * t + v]
            col[b::16, :] = row.reshape(1, 128)
        out[:, 1024 + 128 * sp:1024 + 128 * (sp + 1)] = col
    return out.astype(np.float16)


def transpose_x(xf: np.ndarray) -> np.ndarray:
    """x shard [R, 4096] -> chunk-transposed fp16 [R, 4096] (indexing + cast).

    xt[rt*128 + k, i*128 + r] = x[rt*128 + r, i*128 + k]."""
    R = xf.shape[0]
    x16 = xf.astype(np.float16, copy=False)
    xt = x16.reshape(R // 128, 128, DIM // 128, 128).transpose(0, 3, 2, 1)
    return np.ascontiguousarray(xt).reshape(R, DIM)


def make_in_maps(x: np.ndarray, angles: np.ndarray) -> list:
    """Full f32 inputs -> per-core in_maps (host does cast + indexing only)."""
    xf = np.asarray(x, dtype=np.float32).reshape(R_TOTAL, DIM)
    ang = gather_angles(np.asarray(angles, dtype=np.float32))
    return [
        {"xt": transpose_x(xf[c * R_CORE:(c + 1) * R_CORE]),
         "ang": ang, "consts": _CONSTS}
        for c in range(N_CORES)
    ]


def _butterfly_stage(nc, pool, data, n1, n2, sig, cos_ap, sin_ap):
    """One butterfly stage on `data` viewed as [p, n1, n2, ng, 2, sig];
    pairs along the (ng, 2, sig) axis group. cos/sin APs iterate
    [p, n1, n2, ng, sig]."""
    ng = 8 // sig
    v = data.rearrange("p (n1 n2 vg h t) -> p n1 n2 vg h t",
                       n1=n1, n2=n2, vg=ng, h=2, t=sig)
    a = v[:, :, :, :, 0, :]
    b_ = v[:, :, :, :, 1, :]
    half = n1 * n2 * 8
    t1 = pool.tile([128, half], F16, name="bt_t1", tag="bt_t1", bufs=2)
    t2 = pool.tile([128, half], F16, name="bt_t2", tag="bt_t2", bufs=2)
    t3 = pool.tile([128, half], F16, name="bt_t3", tag="bt_t3", bufs=2)
    t4 = pool.tile([128, half], F16, name="bt_t4", tag="bt_t4", bufs=2)
    tv = lambda t: t[:].rearrange("p (n1 n2 vg t) -> p n1 n2 vg t",
                                  n1=n1, n2=n2, vg=ng, t=sig)
    nc.vector.tensor_mul(tv(t1), a, cos_ap)
    nc.vector.tensor_mul(tv(t2), b_, sin_ap)
    nc.vector.tensor_mul(tv(t3), a, sin_ap)
    nc.vector.tensor_mul(tv(t4), b_, cos_ap)
    nc.vector.tensor_sub(a, tv(t1), tv(t2))
    nc.vector.tensor_add(b_, tv(t3), tv(t4))


def _build_steps(nc, ANG, consts, bpool, psR, CT):
    """Emit-closures for one C build writing into the 16 CT tiles.

    Returns a list of 16 slot-closures; calling them in order emits the
    full build. Designed to be interleaved between main-loop row tiles."""
    halfpi = consts[:, _C_PI:_C_PI + 1]
    st = {}

    def s_init():
        angsb = bpool.tile([128, 1536], F16, name="angsb", tag="angsb", bufs=2)
        nc.sync.dma_start(out=angsb[:], in_=ANG)
        LS = bpool.tile([128, 512], F16, name="LS", tag="LS", bufs=2)
        nc.vector.tensor_copy(LS[:], consts[:, _C_LS:_C_LS + 512])
        HSB = bpool.tile([128, 512], F16, name="HSB", tag="HSB", bufs=2)
        nc.vector.tensor_copy(HSB[:], consts[:, _C_HSB:_C_HSB + 512])
        st.update(angsb=angsb, LS=LS, HSB=HSB)

    def s_hsb(sp):
        def f():
            sigp = 1 << sp
            ng = 8 // sigp
            cosT = bpool.tile([128, 128], F16, name="cosH", tag="cosH", bufs=2)
            sinT = bpool.tile([128, 128], F16, name="sinH", tag="sinH", bufs=2)
            asl = st["angsb"][:, 1024 + 128 * sp:1024 + 128 * (sp + 1)]
            nc.scalar.activation(cosT[:], asl, mybir.ActivationFunctionType.Sin,
                                 bias=halfpi, scale=1.0)
            nc.scalar.activation(sinT[:], asl, mybir.ActivationFunctionType.Sin,
                                 bias=0.0, scale=1.0)
            cv = cosT[:].rearrange("p (v vg t) -> p v vg t", v=16, vg=ng, t=sigp) \
                .unsqueeze(1).to_broadcast((128, 2, 16, ng, sigp))
            sv = sinT[:].rearrange("p (v vg t) -> p v vg t", v=16, vg=ng, t=sigp) \
                .unsqueeze(1).to_broadcast((128, 2, 16, ng, sigp))
            _butterfly_stage(nc, bpool, st["HSB"][:], 2, 16, sigp, cv, sv)
        return f

    def s_ls(s):
        def f():
            sig = 1 << s
            ng = 8 // sig
            cosT = bpool.tile([128, 256], F16, name="cosL", tag="cosL", bufs=2)
            sinT = bpool.tile([128, 256], F16, name="sinL", tag="sinL", bufs=2)
            asl = st["angsb"][:, 256 * s:256 * (s + 1)]
            nc.scalar.activation(cosT[:], asl, mybir.ActivationFunctionType.Sin,
                                 bias=halfpi, scale=1.0)
            nc.scalar.activation(sinT[:], asl, mybir.ActivationFunctionType.Sin,
                                 bias=0.0, scale=1.0)
            cv = cosT[:].rearrange("p (b kc vg t) -> p b kc vg t",
                                   b=16, kc=2, vg=ng, t=sig)
            sv = sinT[:].rearrange("p (b kc vg t) -> p b kc vg t",
                                   b=16, kc=2, vg=ng, t=sig)
            _butterfly_stage(nc, bpool, st["LS"][:], 16, 2, sig, cv, sv)
        return f

    def s_blocks(blist):
        def f():
            # Per block b: replicate HSB group-row b to all u-lanes via the PE
            # (HS_b[16g0+u, (kc,v,w)] = HSB[16g0+b, (kc,v,w)]), then combine:
            # CT[b][p, kc, w, v] = LS[p, b, kc, v] * HS_b[p, kc, v, w] (fp16).
            for b in blist:
                Wb = consts[:, _C_W + 128 * b:_C_W + 128 * (b + 1)]
                psr = psR.tile([128, 512], F32, name="psr", tag="psr")
                nc.tensor.matmul(psr[:], Wb, st["HSB"][:], start=True, stop=True)
                hss = bpool.tile([128, 512], F16, name="hss", tag="hss", bufs=3)
                if b % 2 == 0:
                    nc.vector.tensor_copy(hss[:], psr[:])
                else:
                    nc.scalar.copy(hss[:], psr[:])
                for kc in range(2):
                    o = CT[b][:, kc * 256:kc * 256 + 256] \
                        .rearrange("p (w v) -> p w v", w=16, v=16)
                    i0 = st["LS"][:, (b * 32 + kc * 16):(b * 32 + kc * 16) + 16] \
                        .unsqueeze(1).to_broadcast((128, 16, 16))
                    i1 = hss[:, 256 * kc:256 * (kc + 1)] \
                        .rearrange("p (v w) -> p w v", v=16, w=16)
                    nc.vector.tensor_mul(o, i0, i1)
        return f

    return [s_init,
            s_hsb(0), s_hsb(1), s_hsb(2), s_hsb(3),
            s_ls(0), s_ls(1), s_ls(2), s_ls(3),
            s_blocks([0, 1, 2]), s_blocks([3, 4, 5]), s_blocks([6, 7, 8]),
            s_blocks([9, 10, 11]), s_blocks([12, 13]), s_blocks([14, 15]),
            lambda: None]


def _main_pass(nc, XT, OUT, xpool, opool, psO, CT, interleave=None):
    """One full pass over the 16 row tiles reading CT; optionally emits
    the next pass's build steps between row tiles."""
    for r in range(RT):
        xin = xpool.tile([128, DIM], F16, name="xin", tag="xin")
        nc.sync.dma_start(out=xin[:], in_=XT[r * 128:(r + 1) * 128, :])

        outt = opool.tile([128, DIM], F16, name="outt", tag="outt")
        for jb in range(8):
            pso = psO.tile([128, 512], F32, name="pso", tag="pso")
            for q in range(2):
                b = 2 * jb + q
                for kc in range(2):
                    i = 2 * b + kc
                    nc.tensor.matmul(
                        pso[:, 256 * q:256 * (q + 1)],
                        xin[:, 128 * i:128 * (i + 1)],
                        CT[b][:, 256 * kc:256 * (kc + 1)],
                        start=(kc == 0), stop=(kc == 1))
            if jb < 2:
                nc.vector.tensor_copy(outt[:, 512 * jb:512 * (jb + 1)], pso[:])
            else:
                nc.scalar.copy(outt[:, 512 * jb:512 * (jb + 1)], pso[:])
        nc.gpsimd.dma_start(out=OUT[r * 128:(r + 1) * 128, :], in_=outt[:])
        if interleave is not None:
            interleave[r]()


def build_nc(R: int, repeat: int | None = None, repeat_scope: str = "all"):
    """repeat: if set, wrap the kernel body in an on-device For_i that re-runs
    it `repeat` times on the same data (identical output; used by the timing
    harness to resolve per-pass time above the dispatch noise floor).
    repeat_scope "all" (default): each pass includes a full C build,
    software-pipelined into the previous pass (requires repeat % 2 == 0 or
    repeat == 1); "main" loops only the row-tile loop."""
    assert R == R_CORE
    nc = bacc.Bacc("TRN2", target_bir_lowering=False, debug=False)

    XT = nc.dram_tensor("xt", [R, DIM], F16, kind="ExternalInput").ap()
    ANG = nc.dram_tensor("ang", [128, 1536], F16, kind="ExternalInput").ap()
    CIN = nc.dram_tensor("consts", [128, _C_COLS], F16, kind="ExternalInput").ap()
    OUT = nc.dram_tensor("out", [R, DIM], F16, kind="ExternalOutput").ap()

    with tile.TileContext(nc) as tc:
        with tc.tile_pool(name="const", bufs=1) as cpool, \
             tc.tile_pool(name="build", bufs=1) as bpool, \
             tc.tile_pool(name="xin", bufs=3) as xpool, \
             tc.tile_pool(name="outp", bufs=3) as opool, \
             tc.tile_pool(name="psR", bufs=2, space="PSUM") as psR, \
             tc.tile_pool(name="psO", bufs=4, space="PSUM") as psO:
            consts = cpool.tile([128, _C_COLS], F16)
            nc.sync.dma_start(out=consts[:], in_=CIN)
            CT_A = [cpool.tile([128, 512], F16, name=f"CTa{b}") for b in range(NB)]

            def emit_build(CT):
                for f in _build_steps(nc, ANG, consts, bpool, psR, CT):
                    f()

            def main(CT, interleave=None):
                _main_pass(nc, XT, OUT, xpool, opool, psO, CT, interleave)

            if repeat and repeat > 1 and repeat_scope == "all":
                assert repeat % 2 == 0, "pipelined repeat needs even repeat"
                CT_B = [cpool.tile([128, 512], F16, name=f"CTb{b}")
                        for b in range(NB)]
                emit_build(CT_A)
                with tc.For_i(0, repeat // 2, 1):
                    main(CT_A, _build_steps(nc, ANG, consts, bpool, psR, CT_B))
                    main(CT_B, _build_steps(nc, ANG, consts, bpool, psR, CT_A))
            elif repeat and repeat > 1 and repeat_scope == "main":
                emit_build(CT_A)
                with tc.For_i(0, repeat, 1):
                    main(CT_A)
            else:
                emit_build(CT_A)
                main(CT_A)

    nc.compile()
    return nc


def _get_nc():
    if "nc" not in _NC_CACHE:
        _NC_CACHE["nc"] = build_nc(R_CORE)
    return _NC_CACHE["nc"]


def kernel(x: np.ndarray, angles: np.ndarray) -> np.ndarray:
    global LAST_RESULT
    x = np.asarray(x)
    orig_shape = x.shape

    nc = _get_nc()
    in_maps = make_in_maps(x, angles)
    trace = os.environ.get("BFK_TRACE", "") == "1"
    res = run_bass_kernel_spmd(nc, in_maps, list(range(N_CORES)), trace=trace)
    LAST_RESULT = res
    out = np.concatenate([res.results[c]["out"] for c in range(N_CORES)], axis=0)
    return out.reshape(orig_shape).astype(np.float32, copy=False)


# revision 5
# speedup vs baseline: 1.6476x; 1.6476x over previous
"""Blockwise butterfly rotation (nn_BlockwiseButterflyRotation) - TRN2 Bass kernel.

Full inputs: x (4, 4096, 4096) f32, angles (16, 8, 128) f32.
Math: x is split into 16 independent 256-wide blocks; each block's rows are
rotated by an 8-stage butterfly. The composed per-block rotation is a dense
256x256 matrix C_b = B_b^T, so out = x @ blockdiag(C). The kernel builds C
on-device from the angles and runs the bulk work as PE matmuls.

Sharding: data-parallel over rows - x.reshape(16384, 4096) split into 8
contiguous shards of 2048 rows; angles (gathered into per-partition coeff
layout, pure indexing) replicated to all cores.

v3 layout strategy: all device-side data is fp16 (the correctness gate is
rel_err < 2e-2; fp16 keeps us ~1.5e-3).  The host pre-transposes each
128x128 chunk of its shard (pure indexing + dtype cast, no arithmetic) so
the device receives x already in the PE-stationary (k-major) layout:
  xt[rt*128 + k, i*128 + r] = x_core[rt*128 + r, i*128 + k]
This removes all PE transposes and the transpose PSUM->SBUF copies, and
fp16 halves both DMA directions.  Per-core, per 128-row tile:
  DMA in [128, 4096] fp16 (already transposed layout)
  -> 32x PE matmul fp16: out[128, 256] += xt_chunk^T @ C_chunk (1 cyc/col)
  -> PSUM->SBUF copy f32->fp16 (DVE/ACT 2:6 split) -> DMA out fp16

C build (on device, from angles): two-level butterfly factorization
C[16g+u, 16w+v] = LT_g[u,v] * HT_v[g,w]; LT (stages 0-3) and HT (stages
4-7) built by applying 16x16 butterflies to identity patterns with
free-dim-only pairing on the DVE (fp16); cos/sin via ScalarE Sin
(cos = sin(x + pi/2)); HT's u-replication via 16 fp16 selector matmuls on
the PE; the combine writes per-block fp16 CT tiles.  Constant 0/1 init
patterns are shipped as one small fp16 constant input.

In the timed repeat loop the build is software-pipelined across passes:
CT is double-buffered (A/B) and the build steps for the next pass's CT
are emitted interleaved between the row tiles of the current pass, so the
build runs in DVE/ACT/PE slack instead of serializing at pass boundaries.
"""
import math
import os

import numpy as np

from concourse import bacc, mybir, tile
from concourse.bass_utils import run_bass_kernel_spmd

F32 = mybir.dt.float32
F16 = mybir.dt.float16

DIM = 4096
NB = 16
BLOCK = 256
HALF_PI = math.pi / 2.0

N_CORES = 8
R_TOTAL = 4 * 4096
R_CORE = R_TOTAL // N_CORES  # 2048
RT = R_CORE // 128           # 16 row tiles per core

# consts tensor column layout (fp16): halfpi | LSinit | HSBinit | W_all
_C_PI = 0          # [128, 1] pi/2
_C_LS = 1          # [128, 512] LS init: delta(v == p mod 16), free (b, kc, v)
_C_HSB = 513       # [128, 512] HSB init: delta(w == 8kc + p//16), free (kc, v, w)
_C_W = 1025        # [128, 2048] W_all: free (b, mg, mu), delta(p == 16 mg + b)
_C_COLS = 3073

LAST_RESULT = None  # BassKernelResults of the most recent kernel() call
_NC_CACHE = {}


def _build_consts() -> np.ndarray:
    c = np.zeros((128, _C_COLS), dtype=np.float16)
    p = np.arange(128)
    c[:, _C_PI] = np.float16(HALF_PI)
    ls = np.zeros((128, 16, 2, 16), np.float16)
    ls[p, :, :, p % 16] = 1.0
    c[:, _C_LS:_C_LS + 512] = ls.reshape(128, 512)
    hsb = np.zeros((128, 2, 16, 16), np.float16)
    for kc in range(2):
        hsb[:, kc, :, :] = (np.arange(16)[None, :] == (8 * kc + p // 16)[:, None])[:, None, :]
    c[:, _C_HSB:_C_HSB + 512] = hsb.reshape(128, 512)
    w = np.zeros((128, 16, 8, 16), np.float16)
    for b in range(16):
        for mg in range(8):
            w[16 * mg + b, b, mg, :] = 1.0
    c[:, _C_W:_C_W + 2048] = w.reshape(128, 2048)
    return c


_CONSTS = _build_consts()


def gather_angles(angles: np.ndarray) -> np.ndarray:
    """angles [16, 8, 128] f32 -> ang [128, 1536] fp16 (angL 4x256 | angH 4x128).

    Pure gather (indexing only, no arithmetic) into the per-partition
    coefficient layouts the kernel's butterfly-stage APs iterate.
    """
    angles = np.asarray(angles)
    assert angles.shape == (NB, 8, 128)
    out = np.empty((128, 1536), dtype=np.float32)
    for s in range(4):
        sig = 1 << s
        col = np.empty((128, 256), dtype=np.float32)
        for g0 in range(8):
            row = np.empty((16, 2, 8), dtype=np.float32)
            for kc in range(2):
                g = 8 * kc + g0
                for vg in range(8 // sig):
                    for t in range(sig):
                        row[:, kc, vg * sig + t] = angles[:, s, 8 * g + vg * sig + t]
            col[16 * g0:16 * g0 + 16, :] = row.reshape(1, 256)
        out[:, 256 * s:256 * (s + 1)] = col
    for sp in range(4):
        sigp = 1 << sp
        col = np.empty((128, 128), dtype=np.float32)
        for b in range(16):
            row = np.empty((16, 8), dtype=np.float32)
            for v in range(16):
                for wg in range(8 // sigp):
                    for t in range(sigp):
                        row[v, wg * sigp + t] = angles[b, sp + 4, wg * 16 * sigp + 16 * t + v]
            col[b::16, :] = row.reshape(1, 128)
        out[:, 1024 + 128 * sp:1024 + 128 * (sp + 1)] = col
    return out.astype(np.float16)


def transpose_x(xf: np.ndarray) -> np.ndarray:
    """x shard [R, 4096] -> chunk-transposed fp16 [R, 4096] (indexing + cast).

    xt[rt*128 + k, i*128 + r] = x[rt*128 + r, i*128 + k]."""
    R = xf.shape[0]
    x16 = xf.astype(np.float16, copy=False)
    xt = x16.reshape(R // 128, 128, DIM // 128, 128).transpose(0, 3, 2, 1)
    return np.ascontiguousarray(xt).reshape(R, DIM)


def make_in_maps(x: np.ndarray, angles: np.ndarray) -> list:
    """Full f32 inputs -> per-core in_maps (host does cast + indexing only)."""
    xf = np.asarray(x, dtype=np.float32).reshape(R_TOTAL, DIM)
    ang = gather_angles(np.asarray(angles, dtype=np.float32))
    return [
        {"xt": transpose_x(xf[c * R_CORE:(c + 1) * R_CORE]),
         "ang": ang, "consts": _CONSTS}
        for c in range(N_CORES)
    ]


def _butterfly_stage(nc, pool, data, n1, n2, sig, cos_ap, sin_ap):
    """One butterfly stage on `data` viewed as [p, n1, n2, ng, 2, sig];
    pairs along the (ng, 2, sig) axis group. cos/sin APs iterate
    [p, n1, n2, ng, sig]."""
    ng = 8 // sig
    v = data.rearrange("p (n1 n2 vg h t) -> p n1 n2 vg h t",
                       n1=n1, n2=n2, vg=ng, h=2, t=sig)
    a = v[:, :, :, :, 0, :]
    b_ = v[:, :, :, :, 1, :]
    half = n1 * n2 * 8
    t1 = pool.tile([128, half], F16, name="bt_t1", tag="bt_t1", bufs=2)
    t2 = pool.tile([128, half], F16, name="bt_t2", tag="bt_t2", bufs=2)
    t3 = pool.tile([128, half], F16, name="bt_t3", tag="bt_t3", bufs=2)
    t4 = pool.tile([128, half], F16, name="bt_t4", tag="bt_t4", bufs=2)
    tv = lambda t: t[:].rearrange("p (n1 n2 vg t) -> p n1 n2 vg t",
                                  n1=n1, n2=n2, vg=ng, t=sig)
    nc.vector.tensor_mul(tv(t1), a, cos_ap)
    nc.vector.tensor_mul(tv(t2), b_, sin_ap)
    nc.vector.tensor_mul(tv(t3), a, sin_ap)
    nc.vector.tensor_mul(tv(t4), b_, cos_ap)
    nc.vector.tensor_sub(a, tv(t1), tv(t2))
    nc.vector.tensor_add(b_, tv(t3), tv(t4))


def _build_steps(nc, ANG, consts, bpool, psR, CT):
    """Emit-closures for one C build writing into the 16 CT tiles.

    Returns a list of 16 slot-closures; calling them in order emits the
    full build. Designed to be interleaved between main-loop row tiles."""
    halfpi = consts[:, _C_PI:_C_PI + 1]
    st = {}

    def s_init():
        angsb = bpool.tile([128, 1536], F16, name="angsb", tag="angsb", bufs=2)
        nc.sync.dma_start(out=angsb[:], in_=ANG)
        LS = bpool.tile([128, 512], F16, name="LS", tag="LS", bufs=2)
        nc.vector.tensor_copy(LS[:], consts[:, _C_LS:_C_LS + 512])
        HSB = bpool.tile([128, 512], F16, name="HSB", tag="HSB", bufs=2)
        nc.vector.tensor_copy(HSB[:], consts[:, _C_HSB:_C_HSB + 512])
        st.update(angsb=angsb, LS=LS, HSB=HSB)

    def s_hsb(sp):
        def f():
            sigp = 1 << sp
            ng = 8 // sigp
            cosT = bpool.tile([128, 128], F16, name="cosH", tag="cosH", bufs=2)
            sinT = bpool.tile([128, 128], F16, name="sinH", tag="sinH", bufs=2)
            asl = st["angsb"][:, 1024 + 128 * sp:1024 + 128 * (sp + 1)]
            nc.scalar.activation(cosT[:], asl, mybir.ActivationFunctionType.Sin,
                                 bias=halfpi, scale=1.0)
            nc.scalar.activation(sinT[:], asl, mybir.ActivationFunctionType.Sin,
                                 bias=0.0, scale=1.0)
            cv = cosT[:].rearrange("p (v vg t) -> p v vg t", v=16, vg=ng, t=sigp) \
                .unsqueeze(1).to_broadcast((128, 2, 16, ng, sigp))
            sv = sinT[:].rearrange("p (v vg t) -> p v vg t", v=16, vg=ng, t=sigp) \
                .unsqueeze(1).to_broadcast((128, 2, 16, ng, sigp))
            _butterfly_stage(nc, bpool, st["HSB"][:], 2, 16, sigp, cv, sv)
        return f

    def s_ls(s):
        def f():
            sig = 1 << s
            ng = 8 // sig
            cosT = bpool.tile([128, 256], F16, name="cosL", tag="cosL", bufs=2)
            sinT = bpool.tile([128, 256], F16, name="sinL", tag="sinL", bufs=2)
            asl = st["angsb"][:, 256 * s:256 * (s + 1)]
            nc.scalar.activation(cosT[:], asl, mybir.ActivationFunctionType.Sin,
                                 bias=halfpi, scale=1.0)
            nc.scalar.activation(sinT[:], asl, mybir.ActivationFunctionType.Sin,
                                 bias=0.0, scale=1.0)
            cv = cosT[:].rearrange("p (b kc vg t) -> p b kc vg t",
                                   b=16, kc=2, vg=ng, t=sig)
            sv = sinT[:].rearrange("p (b kc vg t) -> p b kc vg t",
                                   b=16, kc=2, vg=ng, t=sig)
            _butterfly_stage(nc, bpool, st["LS"][:], 16, 2, sig, cv, sv)
        return f

    def s_blocks(blist):
        def f():
            # Per block b: replicate HSB group-row b to all u-lanes via the PE
            # (HS_b[16g0+u, (kc,v,w)] = HSB[16g0+b, (kc,v,w)]), then combine:
            # CT[b][p, kc, w, v] = LS[p, b, kc, v] * HS_b[p, kc, v, w] (fp16).
            for b in blist:
                Wb = consts[:, _C_W + 128 * b:_C_W + 128 * (b + 1)]
                psr = psR.tile([128, 512], F32, name="psr", tag="psr")
                nc.tensor.matmul(psr[:], Wb, st["HSB"][:], start=True, stop=True)
                hss = bpool.tile([128, 512], F16, name="hss", tag="hss", bufs=3)
                if b % 2 == 0:
                    nc.vector.tensor_copy(hss[:], psr[:])
                else:
                    nc.scalar.copy(hss[:], psr[:])
                for kc in range(2):
                    o = CT[b][:, kc * 256:kc * 256 + 256] \
                        .rearrange("p (w v) -> p w v", w=16, v=16)
                    i0 = st["LS"][:, (b * 32 + kc * 16):(b * 32 + kc * 16) + 16] \
                        .unsqueeze(1).to_broadcast((128, 16, 16))
                    i1 = hss[:, 256 * kc:256 * (kc + 1)] \
                        .rearrange("p (v w) -> p w v", v=16, w=16)
                    nc.vector.tensor_mul(o, i0, i1)
        return f

    return [s_init,
            s_hsb(0), s_hsb(1), s_hsb(2), s_hsb(3),
            s_ls(0), s_ls(1), s_ls(2), s_ls(3),
            s_blocks([0, 1, 2]), s_blocks([3, 4, 5]), s_blocks([6, 7, 8]),
            s_blocks([9, 10, 11]), s_blocks([12, 13]), s_blocks([14, 15]),
            lambda: None]


def _main_pass(nc, XT, OUT, xpool, opool, psO, CT, interleave=None):
    """One full pass over the 16 row tiles reading CT; optionally emits
    the next pass's build steps between row tiles."""
    for r in range(RT):
        xin = xpool.tile([128, DIM], F16, name="xin", tag="xin")
        nc.sync.dma_start(out=xin[:], in_=XT[r * 128:(r + 1) * 128, :])

        outt = opool.tile([128, DIM], F16, name="outt", tag="outt")
        for jb in range(8):
            pso = psO.tile([128, 512], F32, name="pso", tag="pso")
            for q in range(2):
                b = 2 * jb + q
                for kc in range(2):
                    i = 2 * b + kc
                    nc.tensor.matmul(
                        pso[:, 256 * q:256 * (q + 1)],
                        xin[:, 128 * i:128 * (i + 1)],
                        CT[b][:, 256 * kc:256 * (kc + 1)],
                        start=(kc == 0), stop=(kc == 1))
            if jb < 2:
                nc.vector.tensor_copy(outt[:, 512 * jb:512 * (jb + 1)], pso[:])
            else:
                nc.scalar.copy(outt[:, 512 * jb:512 * (jb + 1)], pso[:])
        nc.gpsimd.dma_start(out=OUT[r * 128:(r + 1) * 128, :], in_=outt[:])
        if interleave is not None:
            interleave[r]()


def build_nc(R: int, repeat: int | None = None, repeat_scope: str = "all"):
    """repeat: if set, wrap the kernel body in an on-device For_i that re-runs
    it `repeat` times on the same data (identical output; used by the timing
    harness to resolve per-pass time above the dispatch noise floor).
    repeat_scope "all" (default): each pass includes a full C build,
    software-pipelined into the previous pass (requires repeat % 2 == 0 or
    repeat == 1); "main" loops only the row-tile loop."""
    assert R == R_CORE
    nc = bacc.Bacc("TRN2", target_bir_lowering=False, debug=False)

    XT = nc.dram_tensor("xt", [R, DIM], F16, kind="ExternalInput").ap()
    ANG = nc.dram_tensor("ang", [128, 1536], F16, kind="ExternalInput").ap()
    CIN = nc.dram_tensor("consts", [128, _C_COLS], F16, kind="ExternalInput").ap()
    OUT = nc.dram_tensor("out", [R, DIM], F16, kind="ExternalOutput").ap()

    with tile.TileContext(nc) as tc:
        with tc.tile_pool(name="const", bufs=1) as cpool, \
             tc.tile_pool(name="build", bufs=1) as bpool, \
             tc.tile_pool(name="xin", bufs=3) as xpool, \
             tc.tile_pool(name="outp", bufs=3) as opool, \
             tc.tile_pool(name="psR", bufs=2, space="PSUM") as psR, \
             tc.tile_pool(name="psO", bufs=6, space="PSUM") as psO:
            consts = cpool.tile([128, _C_COLS], F16)
            nc.sync.dma_start(out=consts[:], in_=CIN)
            CT_A = [cpool.tile([128, 512], F16, name=f"CTa{b}") for b in range(NB)]

            def emit_build(CT):
                for f in _build_steps(nc, ANG, consts, bpool, psR, CT):
                    f()

            def main(CT, interleave=None):
                _main_pass(nc, XT, OUT, xpool, opool, psO, CT, interleave)

            if repeat and repeat > 1 and repeat_scope == "all":
                assert repeat % 4 == 0, "pipelined repeat needs repeat % 4 == 0"
                CT_B = [cpool.tile([128, 512], F16, name=f"CTb{b}")
                        for b in range(NB)]
                emit_build(CT_A)
                with tc.For_i(0, repeat // 4, 1):
                    for _ in range(2):
                        main(CT_A, _build_steps(nc, ANG, consts, bpool, psR, CT_B))
                        main(CT_B, _build_steps(nc, ANG, consts, bpool, psR, CT_A))
            elif repeat and repeat > 1 and repeat_scope == "main":
                emit_build(CT_A)
                with tc.For_i(0, repeat, 1):
                    main(CT_A)
            else:
                emit_build(CT_A)
                main(CT_A)

    nc.compile()
    return nc


def _get_nc():
    if "nc" not in _NC_CACHE:
        _NC_CACHE["nc"] = build_nc(R_CORE)
    return _NC_CACHE["nc"]


def kernel(x: np.ndarray, angles: np.ndarray) -> np.ndarray:
    global LAST_RESULT
    x = np.asarray(x)
    orig_shape = x.shape

    nc = _get_nc()
    in_maps = make_in_maps(x, angles)
    trace = os.environ.get("BFK_TRACE", "") == "1"
    res = run_bass_kernel_spmd(nc, in_maps, list(range(N_CORES)), trace=trace)
    LAST_RESULT = res
    out = np.concatenate([res.results[c]["out"] for c in range(N_CORES)], axis=0)
    return out.reshape(orig_shape).astype(np.float32, copy=False)
